# revision 6
# baseline (speedup 1.0000x reference)
"""GRU kernel for Trainium2: T=512, B=128, IN=H=256, 8 NeuronCores.

Strategy (data-parallel): shard batch B=128 -> 16 per core, replicate weights.
Per core:
  Phase 1: gx = Wx @ x precompute (bf16 matmuls), bias folded in during PSUM
           evacuation; the length-mask is folded into the z-gate pre-activation
           as a +BIG logit (sigmoid(BIG) == 1.0 -> h frozen), kept in SBUF fp16.
  Phase 2: 512 sequential GRU steps in transposed layout (gate rows on
           partitions, batch on free dim). Wh matmuls in bf16 accumulate into
           PSUM on top of a bh_n seed (identity-matmul trick). sigmoid/tanh on
           ScalarE, elementwise on VectorE. Every 4 steps the fp32 hidden
           states are PE-transposed back to batch-major and DMA'd out.
"""

import numpy as np
import ml_dtypes

import concourse.bass as bass
import concourse.mybir as mybir
from concourse import bacc
from concourse.tile import TileContext
from concourse.bass_utils import run_bass_kernel_spmd

T, B, IN, H = 512, 128, 256, 256
NCORES = 8
BC = B // NCORES            # 16 batch per core
G3 = 3 * H                  # 768 gate rows
NK = IN // 128              # 2 contraction chunks
NM = G3 // 128              # 6 gate-row tiles (0,1=r; 2,3=z; 4,5=n)
TB = T * BC                 # 8192 (t,b) columns
CH = 512                    # phase-1 column chunk (32 timesteps)
NCH = TB // CH              # 16 chunks
BIG = 30000.0

BF16 = mybir.dt.bfloat16
F16 = mybir.dt.float16
F32 = mybir.dt.float32
AF = mybir.ActivationFunctionType
ALU = mybir.AluOpType

_cache = {}


def _build():
    nc = bacc.Bacc("TRN2", target_bir_lowering=False, debug=False,
                   enable_asserts=False, num_devices=NCORES)

    xT_d = nc.dram_tensor("xT", [IN, TB], BF16, kind="ExternalInput").ap()
    wxT_d = nc.dram_tensor("wxT", [IN, G3], BF16, kind="ExternalInput").ap()
    whT_d = nc.dram_tensor("whT", [H, G3], BF16, kind="ExternalInput").ap()
    bias_d = nc.dram_tensor("bias", [G3, 1], F32, kind="ExternalInput").ap()
    seed_d = nc.dram_tensor("seedn", [2, BC, 128], BF16, kind="ExternalInput").ap()
    i16_d = nc.dram_tensor("i16", [BC, BC], BF16, kind="ExternalInput").ap()
    i128_d = nc.dram_tensor("i128", [128, 128], F32, kind="ExternalInput").ap()
    bm_d = nc.dram_tensor("bigmask", [128, TB], F32, kind="ExternalInput").ap()
    h0f_d = nc.dram_tensor("h0f", [128, 2 * BC], F32, kind="ExternalInput").ap()
    h0b_d = nc.dram_tensor("h0b", [128, 2 * BC], BF16, kind="ExternalInput").ap()
    y_d = nc.dram_tensor("y", [T, BC, H], F32, kind="ExternalOutput").ap()

    with TileContext(nc) as tc:
        with tc.tile_pool(name="const", bufs=1) as cpool:
            # Weights as [k-chunk][m-tile] -> [128,128] bf16 lhsT tiles
            wx_t = [[cpool.tile([128, 128], BF16, tag=f"wx{k}{m}", name=f"wx{k}{m}") for m in range(NM)]
                    for k in range(NK)]
            wh_t = [[cpool.tile([128, 128], BF16, tag=f"wh{k}{m}", name=f"wh{k}{m}") for m in range(NM)]
                    for k in range(NK)]
            for k in range(NK):
                for m in range(NM):
                    nc.sync.dma_start(wx_t[k][m], wxT_d[128 * k:128 * (k + 1), 128 * m:128 * (m + 1)])
                    nc.sync.dma_start(wh_t[k][m], whT_d[128 * k:128 * (k + 1), 128 * m:128 * (m + 1)])
            bias_t = [cpool.tile([128, 1], F32, tag=f"bias{m}", name=f"bias{m}") for m in range(NM)]
            for m in range(NM):
                nc.sync.dma_start(bias_t[m], bias_d[128 * m:128 * (m + 1), :])
            seed_t = [cpool.tile([BC, 128], BF16, tag=f"seed{c}", name=f"seed{c}") for c in range(2)]
            for c in range(2):
                nc.sync.dma_start(seed_t[c], seed_d[c])
            i16_t = cpool.tile([BC, BC], BF16, tag="i16", name="i16")
            nc.sync.dma_start(i16_t, i16_d)
            i128_t = cpool.tile([128, 128], F32, tag="i128", name="i128")
            nc.sync.dma_start(i128_t, i128_d)
            h0f_t = cpool.tile([128, 2 * BC], F32, tag="h0f", name="h0f")
            nc.sync.dma_start(h0f_t, h0f_d)
            h0b_t = cpool.tile([128, 2 * BC], BF16, tag="h0b", name="h0b")
            nc.sync.dma_start(h0b_t, h0b_d)
            # the big gx buffer: [128, m-tile(6) x (t,b)(8192)] fp16
            gx_sb = cpool.tile([128, NM * TB], F16, tag="gx", name="gx")

            # ---------------- Phase 1: gx precompute ----------------
            with tc.tile_pool(name="p1", bufs=3) as p1, \
                 tc.tile_pool(name="p1ps", bufs=1, space="PSUM") as p1ps:
                for j in range(NCH):
                    xt = []
                    for k in range(NK):
                        t_ = p1.tile([128, CH], BF16, tag=f"xt{k}", name=f"xt{k}")
                        nc.sync.dma_start(t_, xT_d[128 * k:128 * (k + 1), CH * j:CH * (j + 1)])
                        xt.append(t_)
                    bm = p1.tile([128, CH], F32, tag="bm", name="bm")
                    nc.sync.dma_start(bm, bm_d[:, CH * j:CH * (j + 1)])
                    ps = [p1ps.tile([128, CH], F32, tag=f"gxps{m}", name=f"gxps{m}") for m in range(NM)]
                    for m in range(NM):
                        nc.tensor.matmul(ps[m], wx_t[0][m], xt[0], start=True, stop=False)
                        nc.tensor.matmul(ps[m], wx_t[1][m], xt[1], start=False, stop=True)
                    for m in range(NM):
                        dst = gx_sb[:, m * TB + CH * j: m * TB + CH * (j + 1)]
                        if m in (0, 1):          # r rows: +bias on ACT
                            nc.scalar.add(dst, ps[m], bias_t[m])
                        elif m in (2, 3):        # z rows: +bias +BIG*(1-mask) on DVE
                            nc.vector.scalar_tensor_tensor(
                                dst, ps[m], bias_t[m], bm, op0=ALU.add, op1=ALU.add)
                        else:                    # n rows: +bias (bx only) on DVE
                            nc.vector.tensor_scalar_add(dst, ps[m], bias_t[m])

            # ---------------- Phase 2: sequential scan ----------------
            # gx view: [128, m(6), tb(8192)]
            gxv = gx_sb.rearrange("p (m tb) -> p m tb", m=NM)
            with tc.tile_pool(name="scan", bufs=2) as sp, \
                 tc.tile_pool(name="ystage", bufs=3) as yp, \
                 tc.tile_pool(name="scanps", bufs=2, space="PSUM") as pp:
                h_f = h0f_t    # fp32 h (folded [128, 2*BC]), chunk c at cols [16c:16c+16]
                h_b = h0b_t    # bf16 copy
                ystage = None
                # matmul order: r tiles, n tiles, z tiles (sigmoid(r) is the
                # critical path; z is consumed last)
                m_order = [0, 1, 4, 5, 2, 3]
                for t in range(T):
                    if t % 4 == 0:
                        ystage = yp.tile([128, 128], F32, tag="ystage", name="ystage")
                    mm = pp.tile([128, 96], F32, tag="mm", name="mm")
                    # seed n-gate cols with bh_n (PE write so accumulation works).
                    # Only the tile's FIRST matmul may use start=True: start=True
                    # clears has_written for the whole PSUM bank, which would
                    # wipe earlier groups' accumulation state. With start=False
                    # the per-element has_written bit gives overwrite-then-
                    # accumulate per column group.
                    nc.tensor.matmul(mm[:, 64:80], seed_t[0], i16_t, start=True, stop=False)
                    nc.tensor.matmul(mm[:, 80:96], seed_t[1], i16_t, start=False, stop=False)
                    for m in m_order:
                        nc.tensor.matmul(mm[:, 16 * m:16 * m + 16], wh_t[0][m],
                                         h_b[:, 0:BC], start=False, stop=False)
                        nc.tensor.matmul(mm[:, 16 * m:16 * m + 16], wh_t[1][m],
                                         h_b[:, BC:2 * BC], start=False, stop=True)
                    urz = pp.tile([128, 64], F32, tag="urz", name="urz")
                    nc.vector.tensor_tensor(
                        urz.rearrange("p (m b) -> p m b", m=4),
                        mm[:, 0:64].rearrange("p (m b) -> p m b", m=4),
                        gxv[:, 0:4, BC * t:BC * (t + 1)], op=ALU.add)
                    r_sb = sp.tile([128, 2 * BC], F32, tag="r", name="r")
                    nc.scalar.activation(r_sb, urz[:, 0:32], AF.Sigmoid)
                    z_sb = sp.tile([128, 2 * BC], F32, tag="z", name="z")
                    nc.scalar.activation(z_sb, urz[:, 32:64], AF.Sigmoid)
                    tn = sp.tile([128, 2 * BC], F32, tag="tn", name="tn")
                    nc.vector.tensor_tensor(tn, mm[:, 64:96], r_sb, op=ALU.mult)
                    un = sp.tile([128, 2 * BC], F32, tag="un", name="un")
                    nc.vector.tensor_tensor(
                        un.rearrange("p (m b) -> p m b", m=2),
                        tn.rearrange("p (m b) -> p m b", m=2),
                        gxv[:, 4:6, BC * t:BC * (t + 1)], op=ALU.add)
                    n_sb = sp.tile([128, 2 * BC], F32, tag="n", name="n")
                    nc.scalar.activation(n_sb, un, AF.Tanh)
                    zc = sp.tile([128, 2 * BC], F32, tag="zc", name="zc")
                    nc.vector.tensor_scalar(zc, z_sb, -1.0, 1.0, op0=ALU.mult, op1=ALU.add)
                    a_sb = sp.tile([128, 2 * BC], F32, tag="a", name="a")
                    nc.vector.tensor_tensor(a_sb, z_sb, h_f, op=ALU.mult)
                    b_sb = sp.tile([128, 2 * BC], F32, tag="b", name="b")
                    nc.vector.tensor_tensor(b_sb, zc, n_sb, op=ALU.mult)
                    h_b = sp.tile([128, 2 * BC], BF16, tag="hb", name="hb")
                    nc.vector.tensor_tensor(h_b, a_sb, b_sb, op=ALU.add)
                    h_f = ystage[:, 32 * (t % 4): 32 * (t % 4) + 32]
                    nc.vector.tensor_tensor(h_f, a_sb, b_sb, op=ALU.add)
                    if t % 4 == 3:
                        ytr = pp.tile([128, 128], F32, tag="ytr", name="ytr")
                        nc.tensor.transpose(ytr, ystage, i128_t)
                        yout = yp.tile([128, 128], F32, tag="yout", name="yout")
                        nc.scalar.copy(yout, ytr)
                        t0 = t - 3
                        for ti in range(4):
                            for c in range(2):
                                nc.sync.dma_start(
                                    y_d[t0 + ti, :, 128 * c:128 * (c + 1)],
                                    yout[32 * ti + BC * c: 32 * ti + BC * (c + 1), :])
    nc.compile()
    return nc


def _host_inputs(x, h0, Wx, Wh, bx, bh, length):
    bf = ml_dtypes.bfloat16
    wxT = np.ascontiguousarray(Wx.T).astype(bf)           # [IN, 3H]
    whT = np.ascontiguousarray(Wh.T).astype(bf)           # [H, 3H]
    bias = np.concatenate([(bx + bh)[:2 * H], bx[2 * H:]]).astype(np.float32).reshape(G3, 1)
    seed = np.stack([np.tile(bh[2 * H + 128 * c: 2 * H + 128 * (c + 1)], (BC, 1))
                     for c in range(2)]).astype(bf)       # [2, BC, 128]
    i16 = np.eye(BC, dtype=np.float32).astype(bf)
    i128 = np.eye(128, dtype=np.float32)
    tidx = np.arange(T, dtype=np.int64)

    in_maps = []
    for core in range(NCORES):
        bsl = slice(BC * core, BC * (core + 1))
        xc = np.ascontiguousarray(x[:, bsl, :].transpose(2, 0, 1)).reshape(IN, TB)
        lc = length[bsl].astype(np.int64)
        maskc = (tidx[:, None] < lc[None, :])             # [T, BC]
        bm1 = (BIG * (1.0 - maskc.astype(np.float32))).reshape(TB)
        bmc = np.ascontiguousarray(np.broadcast_to(bm1, (128, TB))).astype(np.float32)
        h0c = np.ascontiguousarray(h0[bsl].T)             # [H, BC]
        h0fold = np.concatenate([h0c[:128], h0c[128:]], axis=1).astype(np.float32)
        in_maps.append({
            "xT": xc.astype(bf),
            "wxT": wxT, "whT": whT, "bias": bias, "seedn": seed,
            "i16": i16, "i128": i128, "bigmask": bmc,
            "h0f": h0fold, "h0b": h0fold.astype(bf),
        })
    return in_maps


def kernel(x, h0, Wx, Wh, bx, bh, length):
    x = np.asarray(x, dtype=np.float32)
    h0 = np.asarray(h0, dtype=np.float32)
    Wx = np.asarray(Wx, dtype=np.float32)
    Wh = np.asarray(Wh, dtype=np.float32)
    bx = np.asarray(bx, dtype=np.float32)
    bh = np.asarray(bh, dtype=np.float32)
    length = np.asarray(length)

    if "nc" not in _cache:
        _cache["nc"] = _build()
    nc = _cache["nc"]
    in_maps = _host_inputs(x, h0, Wx, Wh, bx, bh, length)
    res = run_bass_kernel_spmd(nc, in_maps, core_ids=list(range(NCORES)))
    y = np.empty((T, B, H), dtype=np.float32)
    for core in range(NCORES):
        y[:, BC * core:BC * (core + 1), :] = res.results[core]["y"]
    h_n = y[-1:, :, :].copy()
    return y, h_n
